# revision 1
# baseline (speedup 1.0000x reference)
"""Trainium2 Bass kernel for nn_CAModel (sobel-conv + 2-layer MLP + masked residual).

Math per pixel: y = [x, sobel_x(x), sobel_y(x)] (48 ch); h = relu(w0 @ y + b0);
u = w1 @ h; out = x + u * (rand_u > 0.5).

Sharding: pure data-parallel over 8 cores: (batch b, H-half) -> core b*2 + half.
Each core computes [16, 256, 512] of the output from a [258, 16, 514] padded,
row-major x shard.

Device layout: partitions p = grp*16 + c (8 row-groups x 16 channels), free dim
(rows, W). All stencil shifts are free-dim offsets; sobel built from separable
passes: A = Sy x, B = Dy x, GX = Dx A, GY = Sx B. Pack into Y [48, rows*W] via
partition-offset SBUF-SBUF DMA copies; mm1 K=48 f32->PSUM; relu+bias evac
(ACT/DVE alternating) to bf16; mm2 col-tiled (tile_position) K=128 M=16 into a
32-stacked PSUM tile; mask (stride-0 replicated DMA) + residual add; DMA out.
"""
import os
import numpy as np
from contextlib import ExitStack

import concourse.bass as bass
import concourse.bacc as bacc
import concourse.tile as tile
from concourse import mybir

bf16 = mybir.dt.bfloat16
f32 = mybir.dt.float32
Alu = mybir.AluOpType
Act = mybir.ActivationFunctionType

C = 16          # channels
HID = 128
N_CORES = 8


def build_nc(R=256, WP=514, GR=32, reps=1, ablate=()):
    """Build the per-core graph. R: out rows, WP: padded width, GR: rows/group."""
    W = WP - 2                  # out width
    n_grp = R // GR             # row groups (<= 8)
    assert n_grp * GR == R and n_grp * C <= 128
    GRH = GR + 2                # rows incl halo per group
    FG = GRH * WP               # free elems per partition in group tiles
    UT = 4                      # rows per out (u) tile
    n_ut = R // UT
    mrow = min(R, 128)          # mask tile partition count

    nc = bacc.Bacc()
    x_ext = nc.declare_dram_parameter("x", (n_grp * C, GRH, WP), f32,
                                      isOutput=False)
    ru_ext = nc.declare_dram_parameter("ru", (min(R, 128), ((R + 127) // 128) * W),
                                       f32, isOutput=False)
    w0_ext = nc.declare_dram_parameter("w0", (HID, 3 * C), f32, isOutput=False)
    b0_ext = nc.declare_dram_parameter("b0", (HID, 1), f32, isOutput=False)
    w1_ext = nc.declare_dram_parameter("w1", (C, HID), f32, isOutput=False)
    sel_ext = nc.declare_dram_parameter("sel", (4, 128), f32, isOutput=False)
    # junk-padded output: [u-tile, 128, W]; host strips partitions 16..32 of
    # each 32-block (they are zeros from the padded w1T columns)
    out_ext = nc.declare_dram_parameter("out", (n_ut, 128, W), f32, isOutput=True)

    with tile.TileContext(nc) as tc, ExitStack() as ctx:
        const = ctx.enter_context(tc.tile_pool(name="const", bufs=1))
        big = ctx.enter_context(tc.tile_pool(name="big", bufs=1))
        ypool = ctx.enter_context(tc.tile_pool(name="ypool", bufs=2))
        hpool = ctx.enter_context(tc.tile_pool(name="hpool", bufs=4))
        spool = ctx.enter_context(tc.tile_pool(name="spool", bufs=4))
        psum = ctx.enter_context(tc.tile_pool(name="psum", bufs=3, space="PSUM"))
        upsum = ctx.enter_context(tc.tile_pool(name="upsum", bufs=2, space="PSUM"))

        def _body(_it=None):
            # ---- constants ----
            W0T = const.tile([3 * C, HID], bf16, tag="w0t")
            nc.gpsimd.dma_start(W0T[:], w0_ext[:].transpose([1, 0]))  # cast f32->bf16
            # w1T padded to M=32 (cols 16..32 zero) so mm2 fills full 32-part blocks
            W1T = const.tile([HID, 32], bf16, tag="w1t")
            nc.vector.memset(W1T[:], 0.0)
            nc.gpsimd.dma_start(W1T[:, 0:C], w1_ext[:].transpose([1, 0]))
            B0 = const.tile([HID, 1], f32, tag="b0")
            nc.sync.dma_start(B0[:], b0_ext[:])
            SEL = const.tile([4, 128], bf16, tag="sel")
            nc.gpsimd.dma_start(SEL[:], sel_ext[:])

            # ---- mask: ru arrives host-packed [mrow, nhalf*W]; one load, one compare
            nhalf = (R + mrow - 1) // mrow
            RU = const.tile([mrow, nhalf * W], f32, tag="ru")
            nc.sync.dma_start(RU[:], ru_ext[:])
            Mb = const.tile([mrow, nhalf * W], bf16, tag="mb")
            nc.vector.tensor_scalar(Mb[:], RU[:], 0.5, None, Alu.is_gt)

            # ---- group tiles: x arrives host-packed as [(g,c), rows, W] ----
            NP = n_grp * C
            Xb = big.tile([NP, FG], bf16, tag="xb")
            nc.gpsimd.dma_start(Xb[:].rearrange("p (r w) -> p r w", r=GRH), x_ext[:])

            def gv(t):  # group-tile view [NP, rows, W]
                return t[:, :].rearrange("p (r w) -> p r w", r=GRH)

            # derived tiles hold only the GR center rows (no halo);
            # A and B scratches are W-halves so both chains run in parallel
            FC = GR * WP
            WH = WP // 2 + 1
            AB = big.tile([NP, GR * WH], bf16, tag="ab")
            BB = big.tile([NP, GR * WH], bf16, tag="bb")
            GX = big.tile([NP, FC], bf16, tag="gx")
            GY = big.tile([NP, FC], bf16, tag="gy")

            def cv(t):  # center view [NP, GR, WP]
                return t[:, :].rearrange("p (r w) -> p r w", r=GR)

            xv, abv, gxv, gyv = gv(Xb), cv(AB), cv(GX), cv(GY)
            # edge columns (w=0, WP-1) are never read by mm1 but must be defined
            for t in (gxv, gyv):
                nc.gpsimd.memset(t[:, :, 0:1], 0.0)
                nc.gpsimd.memset(t[:, :, WP - 1:WP], 0.0)
            if 'ab' not in ablate:
                pass
            # A = x[r-1] + 2x[r] + x[r+1]  (center row ro <-> x row ro+1)
            if 'ab' not in ablate:
                # per W-half: A = Sy x -> GX = Dx A;  B = Dy x -> GY = Sx B
                for h2 in range(2):
                    w0_ = h2 * (WP // 2 - 1)        # halves overlap by 2 cols
                    wv = slice(w0_, w0_ + WH)
                    av = AB[:, :].rearrange("p (r w) -> p r w", r=GR)
                    bv = BB[:, :].rearrange("p (r w) -> p r w", r=GR)
                    nc.vector.tensor_tensor(av[:, :, :], xv[:, 0:GRH - 2, wv],
                                            xv[:, 2:GRH, wv], Alu.add)
                    nc.vector.scalar_tensor_tensor(av[:, :, :], xv[:, 1:GRH - 1, wv],
                                                   2.0, av[:, :, :],
                                                   Alu.mult, Alu.add)
                    nc.vector.tensor_tensor(gxv[:, :, w0_ + 1:w0_ + WH - 1],
                                            av[:, :, 2:WH], av[:, :, 0:WH - 2],
                                            Alu.subtract)
                    nc.gpsimd.tensor_tensor(bv[:, :, :], xv[:, 2:GRH, wv],
                                            xv[:, 0:GRH - 2, wv], Alu.subtract)
                    nc.gpsimd.tensor_tensor(gyv[:, :, w0_ + 1:w0_ + WH - 1],
                                            bv[:, :, 0:WH - 2], bv[:, :, 2:WH],
                                            Alu.add)
                    nc.vector.scalar_tensor_tensor(gyv[:, :, w0_ + 1:w0_ + WH - 1],
                                                   bv[:, :, 1:WH - 1], 2.0,
                                                   gyv[:, :, w0_ + 1:w0_ + WH - 1],
                                                   Alu.mult, Alu.add)

            # static double-buffered x-stack tiles; junk partitions zeroed once
            XSbufs = []
            for i in range(4):
                t = const.tile([128, W], bf16, tag=f"xs{i}")
                nc.vector.memset(t[:], 0.0)
                XSbufs.append(t)

            # static M4 rotation (8 tiles) so mask copies carry no slot WAR
            M4bufs = []
            for i in range(8):
                m4t = const.tile([4, W], bf16, tag=f"m4_{i}")
                M4bufs.append(m4t)

            # startup observers: tiny DMAs so each HWDGE lane's vector clock
            # observes the early producers (keeps later DMA wait counts <= 2)
            DUMb = const.tile([8, 4], bf16, tag="dumb")
            DUMf = const.tile([8, 4], f32, tag="dumf")
            for di, srcap in enumerate((Mb[0:1, 0:4], GX[0:1, 0:4], GY[0:1, 0:4],
                                        XSbufs[0][0:1, 0:4], XSbufs[1][0:1, 0:4],
                                        XSbufs[2][0:1, 0:4])):
                nc.sync.dma_start(DUMb[di:di + 1, 0:4], srcap)
            nc.sync.dma_start(DUMf[6:7, 0:4], RU[0:1, 0:4])
            nc.sync.dma_start(DUMf[7:8, 0:1], B0[0:1, 0:1])
            # same for the 8 SWDGE lanes (gpsimd-issued DMAs)
            DUMs = const.tile([8, 4], bf16, tag="dums")
            for di, srcap in enumerate((W0T[0:1, 0:4], W1T[0:1, 0:4], SEL[0:1, 0:4],
                                        Xb[0:1, 0:4], Mb[0:1, 0:4], GX[0:1, 0:4],
                                        GY[0:1, 0:4], Xb[1:2, 0:4])):
                nc.gpsimd.dma_start(DUMs[di:di + 1, 0:4], srcap)

            # ---- main loop: per Y-tile of YR rows ----
            YR = 16 if GR % 16 == 0 else GR
            n_yt = R // YR
            for yt in range(n_yt):
                r0 = yt * YR                     # first out row of tile
                g = r0 // GR                     # group (YR divides GR)
                lr = r0 - GR * g + 1             # local row in group tile (+1 halo)
                Y = ypool.tile([3 * C, YR * WP], bf16, tag="y")
                yv = Y[:, :].rearrange("s (r w) -> s r w", r=YR)
                if 'pack' not in ablate:
                    nc.scalar.dma_start(yv[0:C, :, :],
                                        gv(Xb)[g * C:(g + 1) * C, lr:lr + YR, :])
                    for s, st in enumerate((GX, GY)):
                        (nc.sync if s == 0 else nc.scalar).dma_start(
                            yv[(s + 1) * C:(s + 2) * C, :, :],
                            cv(st)[g * C:(g + 1) * C, lr - 1:lr - 1 + YR, :])

                for ut in range(YR // UT):
                    u_ps = upsum.tile([128, W], f32, tag="u")
                    XS = XSbufs[(yt * (YR // UT) + ut) % 4]
                    hsbs = []
                    for k in range(UT):
                        r = ut * UT + k          # local row in Y
                        rr = r0 + r              # global out row
                        h_ps = psum.tile([HID, W], f32, tag="h")
                        if 'mm1' not in ablate:
                            nc.tensor.matmul(h_ps[:], W0T[:], yv[:, r, 1:WP - 1],
                                             start=True, stop=True)
                        h_sb = hpool.tile([HID, W], bf16, tag="h")
                        hsbs.append(h_sb)
                        if 'evac' not in ablate:
                            if rr % 2 == 0:
                                nc.scalar.activation(h_sb[:], h_ps[:], Act.Relu,
                                                     bias=B0[:])
                            else:
                                nc.vector.tensor_scalar(h_sb[:], h_ps[:], B0[:], 0.0,
                                                        Alu.add, Alu.max)
                        # x_stack rows: partition 32k+c <- Xb[g*16+c, row, 1:WP-1]
                        eng = nc.sync if k % 2 == 0 else nc.scalar
                        eng.dma_start(
                            XS[32 * k:32 * k + C, :],
                            gv(Xb)[g * C:(g + 1) * C, lr + r, 1:WP - 1])
                    if 'mm2' not in ablate:
                        for k in range(UT):
                            nc.tensor.matmul(u_ps[32 * k:32 * k + 32, :], W1T[:],
                                             hsbs[k][:], start=True, stop=True,
                                             tile_position=(0, 32 * k))
                    # mask rows for this u-tile -> [4, W] at partition base 0,
                    # then replicate x32 via PE: REP = SEL.T @ M4 (exact 0/1)
                    rr0 = r0 + ut * UT
                    M4 = M4bufs[(rr0 // UT) % 8]
                    if 'out' in ablate:
                        continue
                    nc.scalar.dma_start(
                        M4[:], Mb[rr0 % mrow:rr0 % mrow + UT,
                                  (rr0 // mrow) * W:(rr0 // mrow) * W + W])
                    REP = upsum.tile([128, W], f32, tag="rep")
                    nc.tensor.matmul(REP[:], SEL[:], M4[:], start=True, stop=True)
                    REPs = spool.tile([128, W], bf16, tag="reps")
                    nc.scalar.activation(REPs[:], REP[:], Act.Copy)
                    UM = spool.tile([128, W], bf16, tag="um")
                    nc.vector.scalar_tensor_tensor(UM[:], u_ps[:], 0.0, REPs[:],
                                                   Alu.bypass, Alu.mult)
                    OF = spool.tile([128, W], f32, tag="of")
                    nc.gpsimd.tensor_tensor(OF[:], UM[:], XS[:], Alu.add)
                    oeng = nc.scalar if (rr0 // UT) % 2 else nc.sync
                    oeng.dma_start(out_ext[rr0 // UT, :, :], OF[:])

        if reps > 1:
            with tc.For_i(0, reps, 1):
                _body()
        else:
            _body()
    return nc


_CACHE = {}


def _get_nc():
    if "nc" not in _CACHE:
        nc = build_nc()
        nc.finalize()
        _CACHE["nc"] = nc
    return _CACHE["nc"]


def _shard_inputs(x, w0, b0, w1, rand_u):
    B, _, H, Wf = x.shape
    half = H // 2
    xp = np.pad(x, ((0, 0), (0, 0), (1, 1), (1, 1))).astype(np.float32)
    w0 = np.ascontiguousarray(w0, np.float32)
    b0 = np.ascontiguousarray(b0, np.float32).reshape(HID, 1)
    w1 = np.ascontiguousarray(w1, np.float32)
    sel = np.kron(np.eye(4), np.ones((1, 32))).astype(np.float32)
    in_maps = []
    GR, GRH = 32, 34
    n_grp = half // GR
    for core in range(N_CORES):
        b, hh = divmod(core, 2)
        xs = xp[b, :, hh * half:hh * half + half + 2, :]      # [16, 258, 514]
        xs = xs.transpose(1, 0, 2)                            # [258, 16, 514]
        # group-packed layout [(g,c), GRH, W] with duplicated halo rows
        xg = np.stack([xs[GR * g:GR * g + GRH] for g in range(n_grp)])
        xs = np.ascontiguousarray(
            xg.transpose(0, 2, 1, 3).reshape(n_grp * 16, GRH, xs.shape[2]))
        rus = rand_u[b, 0, hh * half:(hh + 1) * half, :].astype(np.float32)
        nh = (rus.shape[0] + 127) // 128
        ru = np.ascontiguousarray(
            np.concatenate([rus[i * 128:(i + 1) * 128] for i in range(nh)], axis=1))
        in_maps.append({"x": xs, "ru": ru, "w0": w0, "b0": b0, "w1": w1,
                        "sel": sel})
    return in_maps


def _assemble(results, B, H, Wf):
    out = np.empty((B, C, H, Wf), np.float32)
    half = H // 2
    for core, res in enumerate(results):
        b, hh = divmod(core, 2)
        o = res["out"]                                  # [n_ut, 128, W] padded
        n_ut = o.shape[0]
        o = o.reshape(n_ut, 4, 32, o.shape[2])[:, :, :C, :]   # [n_ut, 4, 16, W]
        o = o.reshape(n_ut * 4, C, o.shape[3])                # [256, 16, 512]
        out[b, :, hh * half:(hh + 1) * half, :] = o.transpose(1, 0, 2)
    return out


def kernel(x, w0, b0, w1, rand_u, _trace=False):
    from concourse.bass_utils import run_bass_kernel_spmd
    nc = _get_nc()
    in_maps = _shard_inputs(x, w0, b0, w1, rand_u)
    res = run_bass_kernel_spmd(nc, in_maps, core_ids=list(range(N_CORES)))
    out = _assemble(res.results, x.shape[0], x.shape[2], x.shape[3])
    if _trace:
        return out, res
    return out


def _run_timed(nc, in_maps, iters):
    import time
    import jax
    from concourse import mybir
    from jax.sharding import Mesh, PartitionSpec
    from jax.experimental.shard_map import shard_map
    from concourse import bass2jax
    from concourse.bass2jax import _bass_exec_p

    bass2jax.install_neuronx_cc_hook()

    pname = nc.partition_id_tensor.name if nc.partition_id_tensor else None
    in_names, out_names, out_avals, zero_outs = [], [], [], []
    for alloc in nc.m.functions[0].allocations:
        if not isinstance(alloc, mybir.MemoryLocationSet):
            continue
        name = alloc.memorylocations[0].name
        if alloc.kind == "ExternalInput":
            if name != pname:
                in_names.append(name)
        elif alloc.kind == "ExternalOutput":
            out_names.append(name)
            shape = tuple(alloc.tensor_shape)
            np_dt = mybir.dt.np(alloc.dtype)
            out_avals.append(jax.core.ShapedArray(shape, np_dt))
            zero_outs.append(np.zeros(shape, np_dt))
    n_params = len(in_names)
    all_in = in_names + out_names
    if pname is not None:
        all_in = all_in + [pname]

    def _body(*args):
        operands = list(args)
        if pname is not None:
            operands.append(bass2jax.partition_id_tensor())
        outs = _bass_exec_p.bind(
            *operands, out_avals=tuple(out_avals), in_names=tuple(all_in),
            out_names=tuple(out_names), lowering_input_output_aliases=(),
            sim_require_finite=True, sim_require_nnan=True, nc=nc)
        return tuple(outs)

    devices = jax.devices()[:N_CORES]
    mesh = Mesh(np.asarray(devices), ("core",))
    specs = (PartitionSpec("core"),)
    fn = jax.jit(shard_map(_body, mesh=mesh,
                           in_specs=specs * (n_params + len(out_names)),
                           out_specs=specs * len(out_names), check_rep=False),
                 keep_unused=True)
    concat_in = [np.concatenate([np.asarray(in_maps[c][n]) for c in range(N_CORES)], axis=0)
                 for n in in_names]
    concat_zeros = [np.zeros((N_CORES * z.shape[0], *z.shape[1:]), z.dtype)
                    for z in zero_outs]
    dev_in = [jax.device_put(a) for a in concat_in + concat_zeros]

    outs = fn(*dev_in)
    jax.block_until_ready(outs)
    best = float("inf")
    for _ in range(iters):
        t0 = time.perf_counter()
        outs = fn(*dev_in)
        jax.block_until_ready(outs)
        best = min(best, time.perf_counter() - t0)

    res = [{n: np.asarray(outs[i]).reshape(N_CORES, *out_avals[i].shape)[c]
            for i, n in enumerate(out_names)} for c in range(N_CORES)]
    return res, best


_REPS = 257


def kernel_timed(x, w0, b0, w1, rand_u, iters=8):
    """Returns (out, est_exec_seconds): marginal per-iteration silicon time
    measured as (wall(reps=9) - wall(reps=1)) / 8 on device-resident inputs."""
    in_maps = _shard_inputs(x, w0, b0, w1, rand_u)
    nc1 = _get_nc()
    res, t1 = _run_timed(nc1, in_maps, iters)
    out = _assemble(res, x.shape[0], x.shape[2], x.shape[3])
    if "ncR" not in _CACHE:
        ncR = build_nc(reps=_REPS)
        ncR.finalize()
        _CACHE["ncR"] = ncR
    resR, tR = _run_timed(_CACHE["ncR"], in_maps, iters)
    outR = _assemble(resR, x.shape[0], x.shape[2], x.shape[3])
    assert np.array_equal(out, outR), "reps variant output mismatch"
    est = (tR - t1) / (_REPS - 1)
    print(f"[timing] wall reps=1: {t1*1e6:.0f} us, reps={_REPS}: {tR*1e6:.0f} us"
          f" -> per-iter {est*1e6:.1f} us")
    return out, est


# ---------------- self-test (simulator, tiny geometry) ----------------
def _ref_numpy(x, w0, b0, w1, rand_u):
    # x [C, Hp+?]: full-precision numpy reference of the per-core math
    import numpy as np
    sx = np.array([[-1, 0, 1], [-2, 0, 2], [-1, 0, 1]], np.float32)
    sy = sx.T
    Cc, H, Wf = x.shape
    xp = np.pad(x, ((0, 0), (1, 1), (1, 1)))
    gx = np.zeros_like(x); gy = np.zeros_like(x)
    for dy in range(3):
        for dx in range(3):
            gx += sx[dy, dx] * xp[:, dy:dy + H, dx:dx + Wf]
            gy += sy[dy, dx] * xp[:, dy:dy + H, dx:dx + Wf]
    y = np.concatenate([x, gx, gy], 0).reshape(3 * Cc, -1)    # [48, H*W]
    h = np.maximum(w0 @ y + b0.reshape(-1, 1), 0)
    u = (w1 @ h).reshape(Cc, H, Wf)
    m = (rand_u > 0.5).astype(np.float32)
    return x + u * m


if __name__ == "__main__":
    from concourse.bass_interp import CoreSim
    R, WP, GR = 16, 18, 8
    Wo = WP - 2
    nc = build_nc(R=R, WP=WP, GR=GR)
    nc.finalize()
    sim = CoreSim(nc)
    rng = np.random.default_rng(0)
    x = rng.standard_normal((R + 2, C, WP)).astype(np.float32)
    x[0] = x[-1] = 0.0
    x[:, :, 0] = x[:, :, -1] = 0.0
    n_grp = R // GR
    xg = np.stack([x[GR * g:GR * g + GR + 2] for g in range(n_grp)])
    x_packed = np.ascontiguousarray(
        xg.transpose(0, 2, 1, 3).reshape(n_grp * C, GR + 2, WP))
    ru = rng.random((R, Wo)).astype(np.float32)
    nh = (R + 127) // 128
    mr = min(R, 128)
    ru_packed = np.ascontiguousarray(
        np.concatenate([ru[i * mr:(i + 1) * mr] for i in range(nh)], axis=1))
    w0 = (rng.standard_normal((HID, 3 * C)) * 0.1).astype(np.float32)
    b0 = (rng.standard_normal((HID, 1)) * 0.1).astype(np.float32)
    w1 = (rng.standard_normal((C, HID)) * 0.1).astype(np.float32)
    sel = np.kron(np.eye(4), np.ones((1, 32))).astype(np.float32)
    for n, v in [("x", x_packed), ("ru", ru_packed), ("w0", w0), ("b0", b0),
                 ("w1", w1), ("sel", sel)]:
        sim.tensor(n)[:] = v
    sim.simulate()
    o = np.array(sim.tensor("out"))
    o = o.reshape(o.shape[0], 4, 32, o.shape[2])[:, :, :C, :]
    got = o.reshape(R, C, Wo).transpose(1, 0, 2)               # [C, R, Wo]
    xin = x[1:R + 1, :, 1:WP - 1].transpose(1, 0, 2)           # [C, R, Wo]
    exp = _ref_numpy(xin, w0, b0.ravel(), w1, ru)
    d = got - exp
    rel = np.linalg.norm(d) / np.linalg.norm(exp)
    print("L2 rel err:", rel, "absmax-scale:", np.abs(d).max() / np.abs(exp).max())
    assert rel < 2e-2, "FAIL"
    print("SIM PASS")



# revision 3
# speedup vs baseline: 1.1839x; 1.1839x over previous
"""Trainium2 Bass kernel for nn_CAModel (sobel-conv + 2-layer MLP + masked
residual).

Math per pixel: y = [x, sobel_x(x), sobel_y(x)] (48 ch); h = relu(w0 @ y + b0);
u = w1 @ h; out = x + u * (rand_u > 0.5).

Sharding: pure data-parallel over 8 cores: (batch b, H-half) -> core b*2 + half.
Each core computes a [16, 256, 512] slice of the output.

Design (v3): this environment is DMA-transfer bound (~120 GB/s effective,
all queues serialized), so everything is batched into few, large DMAs:
- mm1 as ONE K=96 matmul per row: Y96 = [A;B | x;B | A;B] where A = Sy x
  (vertical smooth), B = Dy x (vertical diff), computed by 3 DVE ops.
  Horizontal sobel shifts are baked into the pack DMAs via flat column
  offsets (Y stored flat [96, YR*514+2]; block pair at flat offset 1-shift).
  Block pairs are channel-interleaved (partition = 16*pair + 2c + slab) so
  each pack DMA has a plain partition-range destination.
- Residual x from a host-packed DRAM tensor in the stacked [32k+c] layout.
- Mask: ru > 0.5 on DVE, written to a DRAM scratch tile, read back with a
  partition-broadcast (stride-0 DRAM dim) to build replicated mask tiles.
- Output: bf16 [yt, k, 32, tl, w] (junk half of each 32-block stripped on
  host); host converts to f32 and reassembles.
- Evac (PSUM->SBUF relu+bias) in FD=1024 ops split ACT/DVE; mask-multiply
  on DVE; residual add on GpSimd; out DMAs deferred one y-tile to avoid
  head-of-line blocking on the SP DMA queue.
"""
import numpy as np
from contextlib import ExitStack

import concourse.bass as bass
import concourse.bacc as bacc
import concourse.tile as tile
from concourse import mybir

bf16 = mybir.dt.bfloat16
f32 = mybir.dt.float32
Alu = mybir.AluOpType
Act = mybir.ActivationFunctionType

C = 16          # channels
HID = 128
N_CORES = 8
UT = 4          # rows per u-tile


def build_nc(R=256, W=512, GR=32, YR=16, reps=1, ablate=(), cut=0):
    """Per-core graph. R out rows, W out cols, GR rows/group, YR rows/y-tile."""
    WP = W + 2
    n_grp = R // GR
    n_yt = R // YR
    n_ut = R // UT
    UPY = YR // UT                  # u-tiles per y-tile
    XCH = min(8, n_ut)              # u-tiles per XS chunk
    n_xch = n_ut // XCH
    RCH = min(8, n_ut)              # u-tiles per REP tile
    n_rch = n_ut // RCH
    SLAB = (GR + 2) * WP            # supertile slab elems per partition
    mrow = min(R, 128)
    nhalf = R // mrow
    assert n_grp * C <= 128 and YR <= GR and GR % YR == 0 and YR % UT == 0

    nc = bacc.Bacc()
    x_ext = nc.declare_dram_parameter("x", (n_grp * C, GR + 2, WP), bf16,
                                      isOutput=False)
    xs_ext = nc.declare_dram_parameter("xs", (n_xch, 4, 32, XCH, W), bf16,
                                       isOutput=False)
    ru_ext = nc.declare_dram_parameter("ru", (mrow, nhalf * W), f32,
                                       isOutput=False)
    wabc_ext = nc.declare_dram_parameter("wabc", (96, HID), bf16,
                                         isOutput=False)
    w1x4_ext = nc.declare_dram_parameter("w1x4", (HID, 128), bf16,
                                         isOutput=False)
    b0_ext = nc.declare_dram_parameter("b0", (HID, 1), f32, isOutput=False)
    out_ext = nc.declare_dram_parameter("out", (n_yt, 4, 32, UPY, W), bf16,
                                        isOutput=True)

    with tile.TileContext(nc) as tc, ExitStack() as ctx:
        const = ctx.enter_context(tc.tile_pool(name="const", bufs=1))
        big = ctx.enter_context(tc.tile_pool(name="big", bufs=1))
        ypool = ctx.enter_context(tc.tile_pool(name="ypool", bufs=2))
        xspool = ctx.enter_context(tc.tile_pool(name="xspool", bufs=2))
        reppool = ctx.enter_context(tc.tile_pool(name="reppool", bufs=2))
        hpool = ctx.enter_context(tc.tile_pool(name="hpool", bufs=4))
        umpool = ctx.enter_context(tc.tile_pool(name="umpool", bufs=3))
        opool = ctx.enter_context(tc.tile_pool(name="opool", bufs=2))
        psum = ctx.enter_context(tc.tile_pool(name="psum", bufs=3,
                                              space="PSUM"))
        upsum = ctx.enter_context(tc.tile_pool(name="upsum", bufs=2,
                                               space="PSUM"))
        dpool = ctx.enter_context(tc.tile_pool(name="dram", bufs=1,
                                               space="DRAM"))

        def _body(_it=None):
            # ---- constants ----
            WABC = const.tile([96, HID], bf16, tag="wabc")
            nc.sync.dma_start(WABC[:], wabc_ext[:])
            W1 = const.tile([HID, 128], bf16, tag="w1x4")
            nc.sync.dma_start(W1[:], w1x4_ext[:])
            B0 = const.tile([HID, 1], f32, tag="b0")
            nc.sync.dma_start(B0[:], b0_ext[:])

            # ---- mask: compare on DVE, roundtrip through DRAM scratch ----
            RU = const.tile([mrow, nhalf * W], f32, tag="ru")
            nc.sync.dma_start(RU[:], ru_ext[:])
            Mb = const.tile([mrow, nhalf * W], bf16, tag="mb")
            nc.vector.tensor_scalar(Mb[:], RU[:], 0.5, None, Alu.is_gt)
            MD = dpool.tile([R, W], bf16, tag="md")
            nc.sync.dma_start(
                MD[:].rearrange("(h p) w -> p h w", p=mrow),
                Mb[:].rearrange("p (h w) -> p h w", w=W))

            # ---- supertile: slab0 = x (w/ halos), slab1 = A, slab2 = B ----
            # x load and A/B prepass split into two row-chunks so the first
            # y-tiles can start while the second chunk computes
            SUP = big.tile([n_grp * C, 3, GR + 2, WP], bf16, tag="sup")
            HGR = GR // 2
            nc.sync.dma_start(SUP[:, 0, 0:HGR + 2, :], x_ext[:, 0:HGR + 2, :])
            nc.sync.dma_start(SUP[:, 0, HGR + 2:GR + 2, :],
                              x_ext[:, HGR + 2:GR + 2, :])
            if cut < 8:
                for r0, r1 in ((0, HGR), (HGR, GR)):
                    nc.vector.tensor_tensor(SUP[:, 1, 1 + r0:1 + r1, :],
                                            SUP[:, 0, r0:r1, :],
                                            SUP[:, 0, r0 + 2:r1 + 2, :],
                                            Alu.add)
                    nc.vector.scalar_tensor_tensor(
                        SUP[:, 1, 1 + r0:1 + r1, :],
                        SUP[:, 0, r0 + 1:r1 + 1, :], 2.0,
                        SUP[:, 1, 1 + r0:1 + r1, :], Alu.mult, Alu.add)
                    nc.vector.tensor_tensor(SUP[:, 2, 1 + r0:1 + r1, :],
                                            SUP[:, 0, r0 + 2:r1 + 2, :],
                                            SUP[:, 0, r0:r1, :], Alu.subtract)

            evac_i = [0]
            pending_out = []        # deferred output DMAs

            for yt in range(n_yt):
                g = (yt * YR) // GR
                lr = yt * YR - g * GR

                if (yt * UPY) % RCH == 0 and cut < 7:
                    REP = reppool.tile([128, RCH, W], bf16, tag="rep")
                    base = (yt * UPY) * UT      # first out row of window
                    for k in range(4):
                        src = MD[base:base + UT * RCH, :].rearrange(
                            "(t k) w -> k t w", k=4)[k][None]
                        nc.gpsimd.dma_start(
                            REP[32 * k:32 * k + 32, :, :],
                            src.broadcast_to([32, RCH, W]))
                if (yt * UPY) % XCH == 0 and cut < 7:
                    XS = xspool.tile([128, XCH, W], bf16, tag="xs")
                    ch = (yt * UPY) // XCH
                    for k in range(4):
                        nc.gpsimd.dma_start(XS[32 * k:32 * k + 32, :, :],
                                            xs_ext[ch, k])

                # ---- Y pack: flat [96, YR*514+2]; block flat offset 1-shift
                # Y pack: 3 DMAs; block pair interleaved per-channel so the
                # dst is a plain 32-partition slice (partition = 2c + slab,
                # matching the host-interleaved wabc rows)
                if cut >= 6:
                    continue
                Y = ypool.tile([96, YR * WP + 2], bf16, tag="y")
                rows = SUP[g * C:(g + 1) * C, :, lr + 1:lr + 1 + YR, :]
                for bi, (slabs, off, eng) in enumerate(
                        (((1, 3, 1), 2, nc.sync), ((0, 3, 2), 1, nc.scalar),
                         ((1, 3, 1), 0, nc.gpsimd))):
                    s0, s1, st = slabs
                    src = rows[:, s0:s1:st, :, :]
                    dst = Y[32 * bi:32 * bi + 32, off:off + YR * WP]
                    eng.dma_start(dst, src)

                for tl in range(UPY):
                    ut = yt * UPY + tl
                    hsb = []
                    for j in range(2):
                        h2 = psum.tile([HID, 2 * W], f32, tag="h2")
                        for r2 in range(2):
                            r = tl * UT + j * 2 + r2    # local row in y-tile
                            if 'mm1' not in ablate:
                                nc.tensor.matmul(
                                    h2[:, r2 * W:(r2 + 1) * W], WABC[:],
                                    Y[:, 2 + r * WP:2 + r * WP + W],
                                    start=True, stop=True)
                        hs = hpool.tile([HID, 2 * W], bf16, tag="h")
                        hsb.append(hs)
                        if 'evac' in ablate:
                            evac_i[0] += 1
                            continue
                        if evac_i[0] % 5 == 2:
                            nc.vector.tensor_scalar(hs[:], h2[:], B0[:], 0.0,
                                                    Alu.add, Alu.max)
                        else:
                            nc.scalar.activation(hs[:], h2[:], Act.Relu,
                                                 bias=B0[:])
                        evac_i[0] += 1
                    u_ps = upsum.tile([128, W], f32, tag="u")
                    for k in range(4):
                        if 'mm2' in ablate:
                            break
                        nc.tensor.matmul(u_ps[32 * k:32 * k + 32, :],
                                         W1[:, 32 * k:32 * k + 32],
                                         hsb[k // 2][:, (k % 2) * W:
                                                     (k % 2 + 1) * W],
                                         start=True, stop=True,
                                         tile_position=(0, 32 * k))
                    UM = umpool.tile([128, W], bf16, tag="um")
                    if 'um' not in ablate:
                        nc.vector.scalar_tensor_tensor(
                            UM[:], u_ps[:], 0.0, REP[:, ut % RCH, :],
                            Alu.bypass, Alu.mult)
                    if tl == 0:
                        OFY = opool.tile([128, UPY, W], bf16, tag="ofy")
                    if 'of' not in ablate:
                        nc.gpsimd.tensor_tensor(OFY[:, tl, :], UM[:],
                                                XS[:, ut % XCH, :], Alu.add)

                # defer out DMA by one y-tile so its sem wait doesn't stall
                # the SP sequencer ahead of the next y-tile's pack DMAs
                if 'out' in ablate:
                    continue
                pending_out.append((yt, OFY))
                if len(pending_out) > 1:
                    oyt, oOFY = pending_out.pop(0)
                    nc.sync.dma_start(out_ext[oyt], oOFY[:, :, :])
            for oyt, oOFY in pending_out:
                nc.sync.dma_start(out_ext[oyt], oOFY[:, :, :])

        if reps > 1:
            with tc.For_i(0, reps, 1):
                _body()
        else:
            _body()
    return nc


_CACHE = {}


def _get_nc():
    if "nc" not in _CACHE:
        nc = build_nc()
        nc.finalize()
        _CACHE["nc"] = nc
    return _CACHE["nc"]


def _to_bf16(a):
    import jax.numpy as jnp
    return np.asarray(jnp.asarray(a, dtype=jnp.bfloat16))


def _from_bf16(a):
    import jax.numpy as jnp
    return np.asarray(jnp.asarray(a), dtype=np.float32)


def _pack_weights(w0, b0, w1):
    w0 = np.asarray(w0, np.float32)
    w0x, w0gx, w0gy = w0[:, 0:C], w0[:, C:2 * C], w0[:, 2 * C:3 * C]
    def _ilv(a, b):                 # rows 2c = a[c], 2c+1 = b[c]
        return np.stack([a, b], axis=1).reshape(2 * C, HID)
    wabc = np.concatenate([
        _ilv(-w0gx.T, w0gy.T),      # [A|B interleaved] @ shift -1
        _ilv(w0x.T, 2.0 * w0gy.T),  # [x|B] @ shift 0
        _ilv(w0gx.T, w0gy.T),       # [A|B] @ shift +1
    ], axis=0)                      # [96, HID]
    w1x4 = np.zeros((HID, 128), np.float32)
    for k in range(4):
        w1x4[:, 32 * k:32 * k + C] = np.asarray(w1, np.float32).T
    b0 = np.ascontiguousarray(b0, np.float32).reshape(HID, 1)
    return _to_bf16(wabc), _to_bf16(w1x4), b0


def _shard_inputs(x, w0, b0, w1, rand_u, R=256, W=512, GR=32, YR=16):
    B, _, H, Wf = x.shape
    half = H // 2
    n_grp = R // GR
    n_yt = R // YR
    n_ut = R // UT
    XCH = min(8, n_ut)
    n_xch = n_ut // XCH
    mrow = min(R, 128)
    nhalf = R // mrow
    wabc, w1x4, b0p = _pack_weights(w0, b0, w1)
    xp = np.pad(np.asarray(x, np.float32),
                ((0, 0), (0, 0), (1, 1), (1, 1)))
    in_maps = []
    for core in range(N_CORES):
        b, hh = divmod(core, 2)
        xsl = xp[b, :, hh * half:hh * half + half + 2, :]   # [C, R+2, WP]
        xsl = xsl.transpose(1, 0, 2)                        # [R+2, C, WP]
        xg = np.stack([xsl[GR * g:GR * g + GR + 2] for g in range(n_grp)])
        xpk = np.ascontiguousarray(
            xg.transpose(0, 2, 1, 3).reshape(n_grp * C, GR + 2, W + 2))
        # xs: [chunk, k, 32, XCH, W]; c>=16 zero; row = chunk*4*XCH + 4*t + k
        xc = xp[b, :, hh * half + 1:hh * half + 1 + half, 1:1 + W]  # [C,R,W]
        xs = np.zeros((n_xch, 4, 32, XCH, W), np.float32)
        rows = xc.transpose(1, 0, 2).reshape(n_xch, XCH, 4, C, W)
        xs[:, :, :C, :, :] = rows.transpose(0, 2, 3, 1, 4)
        rus = rand_u[b, 0, hh * half:(hh + 1) * half, :].astype(np.float32)
        ru = np.ascontiguousarray(np.concatenate(
            [rus[i * mrow:(i + 1) * mrow] for i in range(nhalf)], axis=1))
        in_maps.append({
            "x": _to_bf16(xpk), "xs": _to_bf16(xs), "ru": ru,
            "wabc": wabc, "w1x4": w1x4, "b0": b0p})
    return in_maps


def _assemble(results, B, H, Wf, R=256, W=512, YR=16):
    out = np.empty((B, C, H, Wf), np.float32)
    half = H // 2
    n_yt = R // YR
    UPY = YR // UT
    for core, res in enumerate(results):
        b, hh = divmod(core, 2)
        o = _from_bf16(res["out"])[:, :, :C]    # [n_yt, 4, C, UPY, W]
        o = o.transpose(0, 3, 1, 2, 4).reshape(R, C, W)  # row=16yt+4tl+k
        out[b, :, hh * half:(hh + 1) * half, :] = o.transpose(1, 0, 2)
    return out


def kernel(x, w0, b0, w1, rand_u, _trace=False):
    from concourse.bass_utils import run_bass_kernel_spmd
    nc = _get_nc()
    in_maps = _shard_inputs(x, w0, b0, w1, rand_u)
    res = run_bass_kernel_spmd(nc, in_maps, core_ids=list(range(N_CORES)))
    out = _assemble(res.results, x.shape[0], x.shape[2], x.shape[3])
    if _trace:
        return out, res
    return out


def _run_timed(nc, in_maps, iters):
    import time
    import jax
    from concourse import mybir
    from jax.sharding import Mesh, PartitionSpec
    from jax.experimental.shard_map import shard_map
    from concourse import bass2jax
    from concourse.bass2jax import _bass_exec_p

    bass2jax.install_neuronx_cc_hook()

    pname = nc.partition_id_tensor.name if nc.partition_id_tensor else None
    in_names, out_names, out_avals, zero_outs = [], [], [], []
    for alloc in nc.m.functions[0].allocations:
        if not isinstance(alloc, mybir.MemoryLocationSet):
            continue
        name = alloc.memorylocations[0].name
        if alloc.kind == "ExternalInput":
            if name != pname:
                in_names.append(name)
        elif alloc.kind == "ExternalOutput":
            out_names.append(name)
            shape = tuple(alloc.tensor_shape)
            np_dt = mybir.dt.np(alloc.dtype)
            out_avals.append(jax.core.ShapedArray(shape, np_dt))
            zero_outs.append(np.zeros(shape, np_dt))
    n_params = len(in_names)
    all_in = in_names + out_names
    if pname is not None:
        all_in = all_in + [pname]

    def _bodyfn(*args):
        operands = list(args)
        if pname is not None:
            operands.append(bass2jax.partition_id_tensor())
        outs = _bass_exec_p.bind(
            *operands, out_avals=tuple(out_avals), in_names=tuple(all_in),
            out_names=tuple(out_names), lowering_input_output_aliases=(),
            sim_require_finite=True, sim_require_nnan=True, nc=nc)
        return tuple(outs)

    devices = jax.devices()[:N_CORES]
    mesh = Mesh(np.asarray(devices), ("core",))
    specs = (PartitionSpec("core"),)
    fn = jax.jit(shard_map(_bodyfn, mesh=mesh,
                           in_specs=specs * (n_params + len(out_names)),
                           out_specs=specs * len(out_names), check_rep=False),
                 keep_unused=True)
    concat_in = [np.concatenate([np.asarray(in_maps[c][n])
                                 for c in range(N_CORES)], axis=0)
                 for n in in_names]
    concat_zeros = [np.zeros((N_CORES * z.shape[0], *z.shape[1:]), z.dtype)
                    for z in zero_outs]
    dev_in = [jax.device_put(a) for a in concat_in + concat_zeros]

    outs = fn(*dev_in)
    jax.block_until_ready(outs)
    best = float("inf")
    for _ in range(iters):
        t0 = time.perf_counter()
        outs = fn(*dev_in)
        jax.block_until_ready(outs)
        best = min(best, time.perf_counter() - t0)

    res = [{n: np.asarray(outs[i]).reshape(N_CORES, *out_avals[i].shape)[c]
            for i, n in enumerate(out_names)} for c in range(N_CORES)]
    return res, best


_REPS = 1025


def kernel_timed(x, w0, b0, w1, rand_u, iters=12):
    """Returns (out, est_exec_seconds): marginal per-iteration silicon time."""
    in_maps = _shard_inputs(x, w0, b0, w1, rand_u)
    nc1 = _get_nc()
    res, t1 = _run_timed(nc1, in_maps, iters)
    out = _assemble(res, x.shape[0], x.shape[2], x.shape[3])
    if "ncR" not in _CACHE:
        ncR = build_nc(reps=_REPS)
        ncR.finalize()
        _CACHE["ncR"] = ncR
    resR, tR = _run_timed(_CACHE["ncR"], in_maps, iters)
    outR = _assemble(resR, x.shape[0], x.shape[2], x.shape[3])
    assert np.array_equal(out, outR), "reps variant output mismatch"
    est = (tR - t1) / (_REPS - 1)
    print(f"[timing] wall reps=1: {t1*1e6:.0f} us, reps={_REPS}: {tR*1e6:.0f} us"
          f" -> per-iter {est*1e6:.1f} us")
    return out, est


# ---------------- self-test (simulator, tiny geometry) ----------------
def _ref_numpy(x, w0, b0, w1, rand_u):
    sx = np.array([[-1, 0, 1], [-2, 0, 2], [-1, 0, 1]], np.float32)
    sy = sx.T
    Cc, H, Wf = x.shape
    xp = np.pad(x, ((0, 0), (1, 1), (1, 1)))
    gx = np.zeros_like(x)
    gy = np.zeros_like(x)
    for dy in range(3):
        for dx in range(3):
            gx += sx[dy, dx] * xp[:, dy:dy + H, dx:dx + Wf]
            gy += sy[dy, dx] * xp[:, dy:dy + H, dx:dx + Wf]
    y = np.concatenate([x, gx, gy], 0).reshape(3 * Cc, -1)
    h = np.maximum(w0 @ y + b0.reshape(-1, 1), 0)
    u = (w1 @ h).reshape(Cc, H, Wf)
    m = (rand_u > 0.5).astype(np.float32)
    return x + u * m


if __name__ == "__main__":
    from concourse.bass_interp import CoreSim
    R, W, GR, YR = 16, 32, 8, 8
    WP = W + 2
    n_grp = R // GR
    n_ut = R // UT
    XCH = min(8, n_ut)
    n_xch = n_ut // XCH
    mrow = min(R, 128)
    nhalf = R // mrow
    nc = build_nc(R=R, W=W, GR=GR, YR=YR)
    nc.finalize()
    sim = CoreSim(nc)
    rng = np.random.default_rng(0)
    xfull = rng.standard_normal((R + 2, C, WP)).astype(np.float32)
    xfull[0] = xfull[-1] = 0.0
    xfull[:, :, 0] = xfull[:, :, -1] = 0.0
    xg = np.stack([xfull[GR * g:GR * g + GR + 2] for g in range(n_grp)])
    x_packed = np.ascontiguousarray(
        xg.transpose(0, 2, 1, 3).reshape(n_grp * C, GR + 2, WP))
    xc = xfull[1:R + 1, :, 1:WP - 1]                   # [R, C, W]
    xs = np.zeros((n_xch, 4, 32, XCH, W), np.float32)
    rows = xc.reshape(n_xch, XCH, 4, C, W)
    xs[:, :, :C, :, :] = rows.transpose(0, 2, 3, 1, 4)
    ru = rng.random((R, W)).astype(np.float32)
    ru_packed = np.ascontiguousarray(np.concatenate(
        [ru[i * mrow:(i + 1) * mrow] for i in range(nhalf)], axis=1))
    w0 = (rng.standard_normal((HID, 3 * C)) * 0.1).astype(np.float32)
    b0 = (rng.standard_normal((HID, 1)) * 0.1).astype(np.float32)
    w1 = (rng.standard_normal((C, HID)) * 0.1).astype(np.float32)
    wabc, w1x4, b0p = _pack_weights(w0, b0.ravel(), w1)
    for n, v in [("x", _to_bf16(x_packed)), ("xs", _to_bf16(xs)),
                 ("ru", ru_packed), ("wabc", wabc), ("w1x4", w1x4),
                 ("b0", b0p)]:
        sim.tensor(n)[:] = v
    sim.simulate()
    o = _from_bf16(np.array(sim.tensor("out")))[:, :, :C]
    UPY = YR // UT
    got = o.transpose(0, 3, 1, 2, 4).reshape(R, C, W).transpose(1, 0, 2)
    xin = xfull[1:R + 1, :, 1:WP - 1].transpose(1, 0, 2)
    exp = _ref_numpy(xin, w0, b0.ravel(), w1, ru)
    d = got - exp
    rel = np.linalg.norm(d) / np.linalg.norm(exp)
    print("L2 rel err:", rel, "absmax-scale:",
          np.abs(d).max() / np.abs(exp).max())
    assert rel < 2e-2, "FAIL"
    print("SIM PASS")
